# revision 41
# baseline (speedup 1.0000x reference)
"""Causal single-head attention (n=8192, d_model=1024, d_head=128) on 8 TRN2 cores.

Sequence-parallel, K/V projection replicated per core (collectives in this
environment have a ~100us floor -- measured -- so sharded K/V + AllGather
loses).  Core c owns query rows {8i + c} (mod-8 interleave -> causal work
balances exactly and all cores run the identical SPMD instruction stream).

Beyond the bf16 pipeline this version:
  - Projects K/V for key chunks 1..15 with fp8e4 DoubleRow matmuls (two
    128-row k-tiles per instruction -> half the PE instructions at the same
    512-col streaming rate).  Weights are pre-scaled x32 on the host so
    W ~ N(0, 1/1024) lands in fp8e4's normal range; the 1/32 is folded into
    the exp() scale (K path) and into W_o (V path).  Chunk 0 (keys 0..511)
    stays bf16: rows with few attended keys dominate max|y|, and the fp8
    projection error there breaks the 2e-2 gate (measured 2.9e-2..5e-2
    all-fp8 vs 1.1e-2 hybrid).  S/PV matmul inputs stay bf16 casts.
  - x streams as fp8 for chunks 1..15 (half the DMA bytes) + one bf16 chunk 0.
  - S tiles live in 2-bank [P,1024] PSUM tiles so each key tile needs ONE
    exp ACTIVATE (the ACT engine stream, exps + semaphore waits, is the
    2nd-tightest resource after the PE).
  - The V X-bar transpose is deferred one pipeline iteration: issued inline
    it waits on the vt cast and head-of-line-blocks the sync engine stream,
    which also carries the chunk DMA triggers (measured 3-7us PE stalls).
  - Epilogue without activation-table thrash: Z per 128-query block via tiny
    zacc^T @ ones matmuls (partition-major Z), one DVE reciprocal_approx_fast,
    y projected unnormalized into a [P,1024] 2-bank PSUM tile then scaled by
    1/Z with one per-partition tensor_scalar (DVE mid-stream, ACT at tail).
    y leaves as one bf16 DMA per 128-row block (host casts back to f32).
  - Head: weights packed into 3 DMA descriptors (each dma_start costs the
    issuing engine ~0.6-0.9us of desc-gen), x chunk 0 split across the two
    HWDGE rings (sync + scalar), xq on the otherwise-idle scalar ring.
"""

import numpy as np

N_CTX = 8192
D_MODEL = 1024
D_HEAD = 128
NCORES = 8
P = 128
KT = D_MODEL // P          # 8 contraction k-tiles
R = N_CTX // NCORES        # 1024 query rows per core
NCH = 16                   # x^T chunks of 512 keys
NJ = N_CTX // P            # 64 key tiles
SW = 32.0                  # host-side W_q/W_k/W_v scale (fp8 denormal dodge)
INV_SQRT_D = float(1.0 / np.sqrt(D_HEAD))
EXP_SCALE = float(INV_SQRT_D / (SW * SW))
PK2 = KT * P + D_MODEL + 16 + 1   # wvb | wo | mask16 | ones pack width

_CACHE = {}


def _build():
    from contextlib import ExitStack

    import concourse.mybir as mybir
    import concourse.tile as tile
    from concourse import bacc

    f32 = mybir.dt.float32
    bf16 = mybir.dt.bfloat16
    fp8 = mybir.dt.float8e4
    Exp = mybir.ActivationFunctionType.Exp
    Copy = mybir.ActivationFunctionType.Copy
    DR = mybir.MatmulPerfMode.DoubleRow

    nc = bacc.Bacc("TRN2", target_bir_lowering=False, debug=False,
                   num_devices=NCORES)

    # host pre-tiled inputs (see _host_in_maps)
    wkbt = nc.dram_tensor("wkbt", [P, KT, P], bf16, kind="ExternalInput")
    wqt = nc.dram_tensor("wqt", [P, KT, P], bf16, kind="ExternalInput")
    w8 = nc.dram_tensor("w8", [P, 3, KT // 2, 2, P], fp8,
                        kind="ExternalInput")
    wp2 = nc.dram_tensor("wp2", [P, PK2], bf16, kind="ExternalInput")
    xt0 = nc.dram_tensor("xt0", [P, KT, 512], bf16, kind="ExternalInput")
    xt8 = nc.dram_tensor("xt8", [NCH - 1, P, KT, 512], fp8,
                         kind="ExternalInput")
    xq8 = nc.dram_tensor("xq8", [P, 2, KT, 512], fp8, kind="ExternalInput")
    xqb = nc.dram_tensor("xqb", [P, KT, 64], bf16, kind="ExternalInput")
    y = nc.dram_tensor("y", [R, D_MODEL], bf16, kind="ExternalOutput")

    with tile.TileContext(nc) as tc, ExitStack() as ctx:
        consts = ctx.enter_context(tc.tile_pool(name="consts", bufs=1))
        xpool = ctx.enter_context(tc.tile_pool(name="xpool", bufs=4))
        vtpool = ctx.enter_context(tc.tile_pool(name="vtpool", bufs=3))
        sepool = ctx.enter_context(tc.tile_pool(name="sepool", bufs=12))
        vpool = ctx.enter_context(tc.tile_pool(name="vpool", bufs=3))
        spool = ctx.enter_context(tc.tile_pool(name="spool", bufs=3, space="PSUM"))
        pacc = ctx.enter_context(tc.tile_pool(name="pacc", bufs=1, space="PSUM"))

        # ---- persistent SBUF ----
        wkb_sb = consts.tile([P, KT, P], bf16, tag="wkb")
        wq_sb = consts.tile([P, KT, P], bf16, tag="wq")
        w8_sb = consts.tile([P, 3, KT // 2, 2, P], fp8, tag="w8")
        wp2_sb = consts.tile([P, PK2], bf16, tag="wp2")
        xq8_sb = consts.tile([P, 2, KT, 512], fp8, tag="xq8")
        xqb_sb = consts.tile([P, KT, 64], bf16, tag="xqb")
        xt0_sb = consts.tile([P, KT, 512], bf16, tag="xt0")
        kT_sb = consts.tile([P, NJ, P], bf16, tag="kT")
        v_sb = consts.tile([P, NJ, P], bf16, tag="v")
        qT_sb = consts.tile([P, R], bf16, tag="qT")
        oTb_sb = consts.tile([P, R], bf16, tag="oTb")
        zinv_sb = consts.tile([P, 8], f32, tag="zinv")
        zacc_sb = consts.tile([P, R], bf16, tag="zacc")

        def wkb(kt):
            return wkb_sb[:, kt]

        def wqb(kt):
            return wq_sb[:, kt]

        def wvb(kt):
            return wp2_sb[:, 128 * kt:128 * kt + 128]

        def wo(dc):
            return wp2_sb[:, 1024 + 512 * dc:1024 + 512 * dc + 512]

        mask_sb = wp2_sb[:, 2048:2064]
        ones_sb = wp2_sb[:, 2064:2065]

        # ---- PSUM accumulators: O^T per query-column half ----
        oT0 = pacc.tile([P, 512], f32, tag="oT0")
        oT1 = pacc.tile([P, 512], f32, tag="oT1")

        # ---- input DMAs: the head is HBM-bound (8 cores pull the same x
        # replica; measured ~100-150GB/s per ring).  Spread the critical
        # first bytes across all three rings, smallest/neediest-first:
        # K0's matmuls need wkb[kt] + xt0[kt] in kt order, V0 needs wvb
        # (the head of the wp2 pack) ----
        def xt0_piece(eng, k0):
            eng.dma_start(
                out=xt0_sb[:, k0:k0 + 2].rearrange("p k c -> p (k c)"),
                in_=xt0[:, k0:k0 + 2].rearrange("p k c -> p (k c)"))

        nc.sync.dma_start(out=wkb_sb[:, 0:4], in_=wkbt[:, 0:4])
        nc.scalar.dma_start(out=wkb_sb[:, 4:8], in_=wkbt[:, 4:8])
        xt0_piece(nc.sync, 0)
        xt0_piece(nc.scalar, 2)
        xt0_piece(nc.gpsimd, 4)
        xt0_piece(nc.gpsimd, 6)
        nc.scalar.dma_start(out=wp2_sb[:, 0:1024], in_=wp2[:, 0:1024])
        nc.sync.dma_start(out=wp2_sb[:, 1024:PK2], in_=wp2[:, 1024:PK2])
        nc.vector.memset(zacc_sb, 0.0)
        # warm the Exp activation table while ACT is otherwise idle (the
        # 1.28us ACT_TABLE_LOAD otherwise fires on the first real exp,
        # mid-pipeline on the attend critical path)
        warm_sb = vpool.tile([1, 1], f32, tag="w", name="warm")
        nc.scalar.activation(warm_sb[0:1, 0:1], ones_sb[0:1, 0:1], Exp)

        def load_chunk(ch):
            # halves land via both DMA paths (gpsimd SWDGE + sync HWDGE);
            # chunks 1..15 are fp8 so a chunk streams in well under 1us
            xt_t = xpool.tile([P, KT, 512], fp8, tag="xt", name=f"xt{ch}")
            nc.sync.dma_start(
                out=xt_t[:, 0:4].rearrange("p k c -> p (k c)"),
                in_=xt8[ch - 1, :, 0:4].rearrange("p k c -> p (k c)"))
            nc.gpsimd.dma_start(
                out=xt_t[:, 4:8].rearrange("p k c -> p (k c)"),
                in_=xt8[ch - 1, :, 4:8].rearrange("p k c -> p (k c)"))
            return xt_t

        vt_by_ch = {}

        def project_chunk0():
            kv = spool.tile([P, 1024], f32, tag="s", name="kv0")
            kps, vps = kv[:, 0:512], kv[:, 512:1024]
            for kt in range(KT):
                nc.tensor.matmul(kps, wkb(kt), xt0_sb[:, kt],
                                 start=(kt == 0), stop=(kt == KT - 1))
            nc.vector.tensor_copy(kT_sb[:, 0:4, :]
                                  .rearrange("p a b -> p (a b)"), kps)
            for kt in range(KT):
                nc.tensor.matmul(vps, wvb(kt), xt0_sb[:, kt],
                                 start=(kt == 0), stop=(kt == KT - 1))
            vt_t = vtpool.tile([P, 512], bf16, tag="vt", name="vt0")
            nc.vector.tensor_copy(vt_t, vps)
            vt_by_ch[0] = vt_t

        def project_chunk(ch, xt_t):
            # fp8 DoubleRow: two k-tiles per instruction, fp32 PSUM accum
            kv = spool.tile([P, 1024], f32, tag="s", name=f"kv{ch}")
            kps, vps = kv[:, 0:512], kv[:, 512:1024]
            for k2 in range(KT // 2):
                nc.tensor.matmul(kps, w8_sb[:, 0, k2],
                                 xt_t[:, 2 * k2:2 * k2 + 2, :],
                                 start=(k2 == 0), stop=(k2 == KT // 2 - 1),
                                 perf_mode=DR)
            nc.vector.tensor_copy(kT_sb[:, 4 * ch:4 * ch + 4, :]
                                  .rearrange("p a b -> p (a b)"), kps)
            for k2 in range(KT // 2):
                nc.tensor.matmul(vps, w8_sb[:, 1, k2],
                                 xt_t[:, 2 * k2:2 * k2 + 2, :],
                                 start=(k2 == 0), stop=(k2 == KT // 2 - 1),
                                 perf_mode=DR)
            vt_t = vtpool.tile([P, 512], bf16, tag="vt", name=f"vt{ch}")
            nc.vector.tensor_copy(vt_t, vps)
            vt_by_ch[ch] = vt_t

        def transpose_chunk(ch):
            # Deferred one pipeline iteration: the transpose's wait on the vt
            # cast would otherwise head-of-line-block the sync engine stream
            # (chunk DMA triggers queue behind it).  Issued an iteration
            # late, the cast is long done and the wait is ~0; the consumer
            # (attend at ch+2) still has a full chunk of slack.
            nc.sync.dma_start(out=v_sb[:, 4 * ch:4 * ch + 4, :],
                              in_=vt_by_ch.pop(ch), transpose=True)

        def project_q(h):
            # Query cols 0..63 (global rows < 512) in bf16 for accuracy;
            # everything beyond via fp8 DoubleRow (errors there are far
            # below the max|y| rows -- verified 7.0e-3 vs the 2e-2 gate).
            qp = spool.tile([P, 1024], f32, tag="s", name=f"qps{h}")
            qps = qp[:, 0:512]
            if h == 0:
                for kt in range(KT):
                    nc.tensor.matmul(qps[:, 0:64], wqb(kt), xqb_sb[:, kt],
                                     start=(kt == 0), stop=False)
                for k2 in range(KT // 2):
                    nc.tensor.matmul(qps[:, 64:512], w8_sb[:, 2, k2],
                                     xq8_sb[:, 0, 2 * k2:2 * k2 + 2, 64:512],
                                     start=False, stop=(k2 == KT // 2 - 1),
                                     perf_mode=DR)
            else:
                for k2 in range(KT // 2):
                    nc.tensor.matmul(qps, w8_sb[:, 2, k2],
                                     xq8_sb[:, 1, 2 * k2:2 * k2 + 2, :],
                                     start=(k2 == 0), stop=(k2 == KT // 2 - 1),
                                     perf_mode=DR)
            nc.vector.tensor_copy(qT_sb[:, 512 * h:512 * h + 512], qps)

        def attend(J):
            m0 = 16 * J
            se = sepool.tile([P, R], bf16, tag="se", name=f"se{J}")
            se_by_J[J] = se
            sps = spool.tile([P, 1024], f32, tag="s", name=f"s{J}")
            if J < 32:
                nc.tensor.matmul(sps[:, m0:512], kT_sb[:, J, :],
                                 qT_sb[:, m0:512], start=True, stop=True)
                nc.tensor.matmul(sps[:, 512:1024], kT_sb[:, J, :],
                                 qT_sb[:, 512:1024], start=True, stop=True)
            else:
                nc.tensor.matmul(sps[:, m0:1024], kT_sb[:, J, :],
                                 qT_sb[:, m0:1024], start=True, stop=True)
            # ONE exp per key tile (reads across both PSUM banks)
            nc.scalar.activation(se[:, m0:1024], sps[:, m0:1024], Exp,
                                 scale=EXP_SCALE)
            nc.vector.tensor_mul(se[:, m0:m0 + 16], se[:, m0:m0 + 16],
                                 mask_sb)

        def attend_pv(J):
            m0 = 16 * J
            se = se_by_J.pop(J)
            if J < 32:
                nc.tensor.matmul(oT0[:, m0:512], v_sb[:, J, :], se[:, m0:512],
                                 start=(J == 0), stop=(J == 31))
                nc.tensor.matmul(oT1[:, :], v_sb[:, J, :], se[:, 512:1024],
                                 start=(J == 0), stop=False)
            else:
                c0 = m0 - 512
                nc.tensor.matmul(oT1[:, c0:512], v_sb[:, J, :],
                                 se[:, m0:1024], start=False, stop=(J == 63))
            with nc.allow_low_precision(reason="Z partials bf16"):
                nc.vector.tensor_add(zacc_sb[:, m0:1024],
                                     zacc_sb[:, m0:1024], se[:, m0:1024])

        def normalize(h):
            # Z per 128-query block: one [128,1] column per tiny matmul
            # zacc[:, blk].T @ ones; then a single DVE fast reciprocal.
            # No activation tables, no broadcast matmul: y is scaled by
            # 1/Z after projection via per-partition tensor_scalar.
            oT = (oT0, oT1)[h]
            c0 = 512 * h
            zp = spool.tile([P, 1024], f32, tag="s", name=f"zps{h}")
            zps = zp[:, 0:512]
            for qt in range(4):
                blk = slice(c0 + 128 * qt, c0 + 128 * qt + 128)
                nc.tensor.matmul(zps[:, qt:qt + 1], zacc_sb[:, blk],
                                 ones_sb, start=(qt == 0), stop=(qt == 3))
            nc.vector.reciprocal_approx_fast(zinv_sb[:, 4 * h:4 * h + 4],
                                             zps[:, 0:4])
            with nc.allow_low_precision(reason="O^T in bf16 for y matmul"):
                nc.vector.tensor_copy(oTb_sb[:, c0:c0 + 512], oT)

        def y_project(g):
            # y rows 128g..128g+127 (unnormalized, both d-halves in one
            # 2-bank PSUM tile), then one scale by 1/Z and one DMA.
            # Half 0 scales on DVE (ACT is mid-exp-stream); half 1 on ACT.
            yp = spool.tile([P, 1024], f32, tag="s", name=f"y{g}")
            for dc in range(2):
                nc.tensor.matmul(yp[:, 512 * dc:512 * dc + 512],
                                 oTb_sb[:, 128 * g:128 * g + 128],
                                 wo(dc), start=True, stop=True)
            y_sb = vpool.tile([P, 1024], bf16, tag="y", name=f"ysb{g}")
            with nc.allow_low_precision(reason="y output bf16"):
                if g < 4:
                    # mid-stream: ACT is busy with exps, scale on DVE
                    nc.vector.tensor_scalar_mul(y_sb, yp,
                                                zinv_sb[:, g:g + 1])
                else:
                    # tail: split halves across DVE + ACT in parallel
                    nc.vector.tensor_scalar_mul(y_sb[:, 0:512],
                                                yp[:, 0:512],
                                                zinv_sb[:, g:g + 1])
                    nc.scalar.activation(y_sb[:, 512:1024], yp[:, 512:1024],
                                         Copy, scale=zinv_sb[:, g:g + 1])
            if g < 4:
                eng = nc.gpsimd      # keep the sync ring free: it carries
            else:                    # the chunk-15 transpose these overlap
                eng = nc.scalar if g % 2 == 0 else nc.sync
            eng.dma_start(out=y[128 * g:128 * g + 128, :], in_=y_sb)

        se_by_J = {}
        # ---- main pipeline: project chunk ch, attend chunk ch-2 ----
        q = [load_chunk(1)]
        nc.sync.dma_start(out=wq_sb, in_=wqt[:, :, :])
        nc.scalar.dma_start(out=xqb_sb, in_=xqb[:, :, :])
        nc.gpsimd.dma_start(out=w8_sb, in_=w8[:, :, :, :, :])
        q.append(load_chunk(2))
        nc.scalar.dma_start(out=xq8_sb[:, 0], in_=xq8[:, 0])
        q.append(load_chunk(3))
        nc.scalar.dma_start(out=xq8_sb[:, 1], in_=xq8[:, 1])
        project_chunk0()
        # attend schedule: lag-2 through ch10, then drain the backlog with
        # 5-7 tiles per iteration so only J=60..63 trail the loop (a single
        # 12-tile iteration at ch15 left the PE idle around its edges)
        SCHED = {2: [0, 1], 3: [2, 3]}
        for c in range(4, 11):
            SCHED[c] = list(range(4 * (c - 3), 4 * (c - 3) + 4))
        SCHED[11] = list(range(32, 37))
        SCHED[12] = list(range(37, 42))
        SCHED[13] = list(range(42, 47))
        SCHED[14] = list(range(47, 53))
        SCHED[15] = list(range(53, 60))

        def attend_pairs(js):
            for i in range(0, len(js), 2):
                pair = js[i:i + 2]
                for J in pair:
                    attend(J)
                for J in pair:
                    attend_pv(J)

        for ch in range(1, NCH):
            xt_t = q.pop(0)
            if ch + 3 < NCH:
                q.append(load_chunk(ch + 3))
            project_chunk(ch, xt_t)
            # chunks <=12: transpose deferred one iteration (head-of-line
            # blocking); chunks >=13: inline (no more chunk loads to block)
            if ch <= 13:
                transpose_chunk(ch - 1)
            if ch >= 13:
                transpose_chunk(ch)
            if ch == 1:
                project_q(0)
                continue
            if ch == 2:
                project_q(1)
            attend_pairs(SCHED[ch])
            if ch == 10:
                normalize(0)
            elif 11 <= ch <= 14:
                y_project(ch - 11)  # half-0 output, spread over chunks
        attend_pairs([60, 61, 62, 63])
        normalize(1)
        for g in range(4, 8):
            y_project(g)

    nc.compile()
    return nc


def _get_nc():
    if "nc" not in _CACHE:
        _CACHE["nc"] = _build()
    return _CACHE["nc"]


def _host_in_maps(x, W_q, W_k, W_v, W_o):
    import ml_dtypes
    bf16 = ml_dtypes.bfloat16
    fp8 = ml_dtypes.float8_e4m3

    x = np.asarray(x, dtype=np.float32)
    xTf = np.ascontiguousarray(x.T)                        # [1024, 8192] f32
    xTk = xTf.reshape(KT, P, N_CTX)                        # [kt, p, col]
    # chunk 0 bf16: xt0[p, kt, j'] = xT[128kt+p, j']
    xt0_t = np.ascontiguousarray(
        xTk[:, :, 0:512].transpose(1, 0, 2).astype(bf16))
    # chunks 1..15 fp8: xt8[ch-1, p, kt, j'] = xT[128kt+p, 512ch + j']
    xt8_t = np.ascontiguousarray(
        xTk.reshape(KT, P, NCH, 512)[:, :, 1:].transpose(2, 1, 0, 3)
        .astype(fp8))

    def wtile(w, scale=1.0):
        # [p, kt, h] = scale * W[128kt + p, h]
        return (np.asarray(w, np.float32) * scale) \
            .reshape(KT, P, D_HEAD).transpose(1, 0, 2)

    def wtile8(w):
        # [p, k2, i, h] = 32*W[256k2 + 128i + p, h]  (fp8)
        return (np.asarray(w, np.float32) * SW).astype(fp8) \
            .reshape(KT // 2, 2, P, D_HEAD).transpose(2, 0, 1, 3)

    wkb_t = np.ascontiguousarray(wtile(W_k, SW).astype(bf16))
    wq_t = np.ascontiguousarray(wtile(W_q, SW).astype(bf16))
    # w8 = wk8 | wv8 | wq8 (all x32 fp8)
    w8_t = np.ascontiguousarray(
        np.stack([wtile8(W_k), wtile8(W_v), wtile8(W_q)], axis=1))
    # wp2 = wvb (x32) flat | wo (/32) | mask16 | ones
    wvb_flat = wtile(W_v, SW).reshape(P, KT * D_HEAD)
    wo_t = np.asarray(W_o, np.float32) / SW                # [128, 1024]
    ones_t = np.ones((P, 1), np.float32)
    pp = np.arange(P)[:, None]
    oo = np.arange(16)[None, :]

    in_maps = []
    for c in range(NCORES):
        # xq[p, ch, kt, m'] = xT[128kt+p, 8(512ch+m') + c]
        xq_c = xTk[:, :, c::NCORES] \
            .reshape(KT, P, 2, 512).transpose(1, 2, 0, 3)
        xq8_c = np.ascontiguousarray(xq_c.astype(fp8))
        xqb_c = np.ascontiguousarray(
            xq_c[:, 0, :, 0:64].astype(bf16))              # [P, KT, 64]
        mask_c = (8 * oo + c >= pp).astype(np.float32)     # [128, 16]
        wp2_t = np.ascontiguousarray(
            np.concatenate([wvb_flat, wo_t, mask_c, ones_t], axis=1)
            .astype(bf16))
        in_maps.append({
            "wkbt": wkb_t, "wqt": wq_t, "w8": w8_t, "wp2": wp2_t,
            "xt0": xt0_t, "xt8": xt8_t, "xq8": xq8_c, "xqb": xqb_c,
        })
    return in_maps


def _run(x, W_q, W_k, W_v, W_o, trace=False):
    from concourse.bass_utils import run_bass_kernel_spmd
    nc = _get_nc()
    in_maps = _host_in_maps(x, W_q, W_k, W_v, W_o)
    res = run_bass_kernel_spmd(nc, in_maps, list(range(NCORES)), trace=trace)
    out = np.empty((N_CTX, D_MODEL), dtype=np.float32)
    for c in range(NCORES):
        out[c::NCORES] = np.asarray(res.results[c]["y"], dtype=np.float32)
    return out, res


def kernel(x, W_q, W_k, W_v, W_o):
    out, _ = _run(x, W_q, W_k, W_v, W_o, trace=False)
    return out


# revision 42
# speedup vs baseline: 1.1428x; 1.1428x over previous
"""Causal single-head attention (n=8192, d_model=1024, d_head=128) on 8 TRN2 cores.

Sequence-parallel, K/V projection replicated per core (collectives in this
environment have a ~100us floor -- measured -- so sharded K/V + AllGather
loses).  Core c owns query rows {8i + c} (mod-8 interleave -> causal work
balances exactly and all cores run the identical SPMD instruction stream).

Beyond the bf16 pipeline this version:
  - Projects K/V for key chunks 1..15 with fp8e4 DoubleRow matmuls (two
    128-row k-tiles per instruction -> half the PE instructions at the same
    512-col streaming rate).  Weights are pre-scaled x32 on the host so
    W ~ N(0, 1/1024) lands in fp8e4's normal range; the 1/32 is folded into
    the exp() scale (K path) and into W_o (V path).  Chunk 0 (keys 0..511)
    stays bf16: rows with few attended keys dominate max|y|, and the fp8
    projection error there breaks the 2e-2 gate (measured 2.9e-2..5e-2
    all-fp8 vs 1.1e-2 hybrid).  S/PV matmul inputs stay bf16 casts.
  - x streams as fp8 for chunks 1..15 (half the DMA bytes) + one bf16 chunk 0.
  - S tiles live in 2-bank [P,1024] PSUM tiles so each key tile needs ONE
    exp ACTIVATE (the ACT engine stream, exps + semaphore waits, is the
    2nd-tightest resource after the PE).
  - The V X-bar transpose is deferred one pipeline iteration: issued inline
    it waits on the vt cast and head-of-line-blocks the sync engine stream,
    which also carries the chunk DMA triggers (measured 3-7us PE stalls).
  - Epilogue without activation-table thrash: Z per 128-query block via tiny
    zacc^T @ ones matmuls (partition-major Z), one DVE reciprocal_approx_fast,
    y projected unnormalized into a [P,1024] 2-bank PSUM tile then scaled by
    1/Z with one per-partition tensor_scalar (DVE mid-stream, ACT at tail).
    y leaves as one bf16 DMA per 128-row block (host casts back to f32).
  - Head: weights packed into 3 DMA descriptors (each dma_start costs the
    issuing engine ~0.6-0.9us of desc-gen), x chunk 0 split across the two
    HWDGE rings (sync + scalar), xq on the otherwise-idle scalar ring.
"""

import numpy as np

N_CTX = 8192
D_MODEL = 1024
D_HEAD = 128
NCORES = 8
P = 128
KT = D_MODEL // P          # 8 contraction k-tiles
R = N_CTX // NCORES        # 1024 query rows per core
NCH = 16                   # x^T chunks of 512 keys
NJ = N_CTX // P            # 64 key tiles
SW = 32.0                  # host-side W_q/W_k/W_v scale (fp8 denormal dodge)
INV_SQRT_D = float(1.0 / np.sqrt(D_HEAD))
EXP_SCALE = float(INV_SQRT_D / (SW * SW))
PK2 = KT * P + D_MODEL + 16 + 1   # wvb | wo | mask16 | ones pack width

_CACHE = {}


def _build():
    from contextlib import ExitStack

    import concourse.mybir as mybir
    import concourse.tile as tile
    from concourse import bacc

    f32 = mybir.dt.float32
    bf16 = mybir.dt.bfloat16
    fp8 = mybir.dt.float8e4
    Exp = mybir.ActivationFunctionType.Exp
    Copy = mybir.ActivationFunctionType.Copy
    DR = mybir.MatmulPerfMode.DoubleRow

    nc = bacc.Bacc("TRN2", target_bir_lowering=False, debug=False,
                   num_devices=NCORES)

    # host pre-tiled inputs (see _host_in_maps)
    wkbt = nc.dram_tensor("wkbt", [P, KT, P], bf16, kind="ExternalInput")
    wqt = nc.dram_tensor("wqt", [P, KT, P], bf16, kind="ExternalInput")
    w8 = nc.dram_tensor("w8", [P, 3, KT // 2, 2, P], fp8,
                        kind="ExternalInput")
    wp2 = nc.dram_tensor("wp2", [P, PK2], bf16, kind="ExternalInput")
    xt0 = nc.dram_tensor("xt0", [P, KT, 512], bf16, kind="ExternalInput")
    xt8 = nc.dram_tensor("xt8", [NCH - 1, P, KT, 512], fp8,
                         kind="ExternalInput")
    xq8 = nc.dram_tensor("xq8", [P, 2, KT, 512], fp8, kind="ExternalInput")
    xqb = nc.dram_tensor("xqb", [P, KT, 64], bf16, kind="ExternalInput")
    y = nc.dram_tensor("y", [R, D_MODEL], bf16, kind="ExternalOutput")

    with tile.TileContext(nc) as tc, ExitStack() as ctx:
        consts = ctx.enter_context(tc.tile_pool(name="consts", bufs=1))
        xpool = ctx.enter_context(tc.tile_pool(name="xpool", bufs=4))
        vtpool = ctx.enter_context(tc.tile_pool(name="vtpool", bufs=3))
        sepool = ctx.enter_context(tc.tile_pool(name="sepool", bufs=12))
        vpool = ctx.enter_context(tc.tile_pool(name="vpool", bufs=3))
        spool = ctx.enter_context(tc.tile_pool(name="spool", bufs=2, space="PSUM"))
        pslo = ctx.enter_context(tc.tile_pool(name="pslo", bufs=1, space="PSUM"))
        pshi = ctx.enter_context(tc.tile_pool(name="pshi", bufs=1, space="PSUM"))
        pacc = ctx.enter_context(tc.tile_pool(name="pacc", bufs=1, space="PSUM"))

        # ---- persistent SBUF ----
        wkb_sb = consts.tile([P, KT, P], bf16, tag="wkb")
        wq_sb = consts.tile([P, KT, P], bf16, tag="wq")
        w8_sb = consts.tile([P, 3, KT // 2, 2, P], fp8, tag="w8")
        wp2_sb = consts.tile([P, PK2], bf16, tag="wp2")
        xq8_sb = consts.tile([P, 2, KT, 512], fp8, tag="xq8")
        xqb_sb = consts.tile([P, KT, 64], bf16, tag="xqb")
        xt0_sb = consts.tile([P, KT, 512], bf16, tag="xt0")
        kT_sb = consts.tile([P, NJ, P], bf16, tag="kT")
        v_sb = consts.tile([P, NJ, P], bf16, tag="v")
        qT_sb = consts.tile([P, R], bf16, tag="qT")
        oTb_sb = consts.tile([P, R], bf16, tag="oTb")
        zinv_sb = consts.tile([P, 8], f32, tag="zinv")
        zacc_sb = consts.tile([P, R], bf16, tag="zacc")

        def wkb(kt):
            return wkb_sb[:, kt]

        def wqb(kt):
            return wq_sb[:, kt]

        def wvb(kt):
            return wp2_sb[:, 128 * kt:128 * kt + 128]

        def wo(dc):
            return wp2_sb[:, 1024 + 512 * dc:1024 + 512 * dc + 512]

        mask_sb = wp2_sb[:, 2048:2064]
        ones_sb = wp2_sb[:, 2064:2065]

        # ---- PSUM accumulators: O^T per query-column half ----
        oT0 = pacc.tile([P, 512], f32, tag="oT0")
        oT1 = pacc.tile([P, 512], f32, tag="oT1")

        # ---- input DMAs: the head is HBM-bound (8 cores pull the same x
        # replica; measured ~100-150GB/s per ring).  Spread the critical
        # first bytes across all three rings, smallest/neediest-first:
        # K0's matmuls need wkb[kt] + xt0[kt] in kt order, V0 needs wvb
        # (the head of the wp2 pack) ----
        def xt0_piece(eng, k0):
            eng.dma_start(
                out=xt0_sb[:, k0:k0 + 2].rearrange("p k c -> p (k c)"),
                in_=xt0[:, k0:k0 + 2].rearrange("p k c -> p (k c)"))

        nc.sync.dma_start(out=wkb_sb[:, 0:4], in_=wkbt[:, 0:4])
        nc.scalar.dma_start(out=wkb_sb[:, 4:8], in_=wkbt[:, 4:8])
        xt0_piece(nc.sync, 0)
        xt0_piece(nc.scalar, 2)
        xt0_piece(nc.gpsimd, 4)
        xt0_piece(nc.gpsimd, 6)
        nc.scalar.dma_start(out=wp2_sb[:, 0:1024], in_=wp2[:, 0:1024])
        nc.sync.dma_start(out=wp2_sb[:, 1024:PK2], in_=wp2[:, 1024:PK2])
        nc.vector.memset(zacc_sb, 0.0)
        # warm the Exp activation table while ACT is otherwise idle (the
        # 1.28us ACT_TABLE_LOAD otherwise fires on the first real exp,
        # mid-pipeline on the attend critical path)
        warm_sb = vpool.tile([1, 1], f32, tag="w", name="warm")
        nc.scalar.activation(warm_sb[0:1, 0:1], ones_sb[0:1, 0:1], Exp)

        def load_chunk(ch):
            # halves land via both DMA paths (gpsimd SWDGE + sync HWDGE);
            # chunks 1..15 are fp8 so a chunk streams in well under 1us
            xt_t = xpool.tile([P, KT, 512], fp8, tag="xt", name=f"xt{ch}")
            nc.sync.dma_start(
                out=xt_t[:, 0:4].rearrange("p k c -> p (k c)"),
                in_=xt8[ch - 1, :, 0:4].rearrange("p k c -> p (k c)"))
            nc.gpsimd.dma_start(
                out=xt_t[:, 4:8].rearrange("p k c -> p (k c)"),
                in_=xt8[ch - 1, :, 4:8].rearrange("p k c -> p (k c)"))
            return xt_t

        vt_by_ch = {}

        def project_chunk0():
            kps = pslo.tile([P, 512], f32, tag="lo", name="kps0")
            vps = pshi.tile([P, 512], f32, tag="hi", name="vps0")
            for kt in range(KT):
                nc.tensor.matmul(kps, wkb(kt), xt0_sb[:, kt],
                                 start=(kt == 0), stop=(kt == KT - 1))
            nc.vector.tensor_copy(kT_sb[:, 0:4, :]
                                  .rearrange("p a b -> p (a b)"), kps)
            for kt in range(KT):
                nc.tensor.matmul(vps, wvb(kt), xt0_sb[:, kt],
                                 start=(kt == 0), stop=(kt == KT - 1))
            vt_t = vtpool.tile([P, 512], bf16, tag="vt", name="vt0")
            nc.vector.tensor_copy(vt_t, vps)
            vt_by_ch[0] = vt_t

        def project_chunk(ch, xt_t):
            # fp8 DoubleRow: two k-tiles per instruction, fp32 PSUM accum
            kps = pslo.tile([P, 512], f32, tag="lo", name=f"kps{ch}")
            vps = pshi.tile([P, 512], f32, tag="hi", name=f"vps{ch}")
            for k2 in range(KT // 2):
                nc.tensor.matmul(kps, w8_sb[:, 0, k2],
                                 xt_t[:, 2 * k2:2 * k2 + 2, :],
                                 start=(k2 == 0), stop=(k2 == KT // 2 - 1),
                                 perf_mode=DR)
            nc.vector.tensor_copy(kT_sb[:, 4 * ch:4 * ch + 4, :]
                                  .rearrange("p a b -> p (a b)"), kps)
            for k2 in range(KT // 2):
                nc.tensor.matmul(vps, w8_sb[:, 1, k2],
                                 xt_t[:, 2 * k2:2 * k2 + 2, :],
                                 start=(k2 == 0), stop=(k2 == KT // 2 - 1),
                                 perf_mode=DR)
            vt_t = vtpool.tile([P, 512], bf16, tag="vt", name=f"vt{ch}")
            nc.vector.tensor_copy(vt_t, vps)
            vt_by_ch[ch] = vt_t

        def transpose_chunk(ch):
            # Deferred one pipeline iteration: the transpose's wait on the vt
            # cast would otherwise head-of-line-block the sync engine stream
            # (chunk DMA triggers queue behind it).  Issued an iteration
            # late, the cast is long done and the wait is ~0; the consumer
            # (attend at ch+2) still has a full chunk of slack.
            nc.sync.dma_start(out=v_sb[:, 4 * ch:4 * ch + 4, :],
                              in_=vt_by_ch.pop(ch), transpose=True)

        def project_q(h):
            # Query cols 0..63 (global rows < 512) in bf16 for accuracy;
            # everything beyond via fp8 DoubleRow (errors there are far
            # below the max|y| rows -- verified 7.0e-3 vs the 2e-2 gate).
            qps = pslo.tile([P, 512], f32, tag="lo", name=f"qps{h}")
            if h == 0:
                for kt in range(KT):
                    nc.tensor.matmul(qps[:, 0:64], wqb(kt), xqb_sb[:, kt],
                                     start=(kt == 0), stop=False)
                for k2 in range(KT // 2):
                    nc.tensor.matmul(qps[:, 64:512], w8_sb[:, 2, k2],
                                     xq8_sb[:, 0, 2 * k2:2 * k2 + 2, 64:512],
                                     start=False, stop=(k2 == KT // 2 - 1),
                                     perf_mode=DR)
            else:
                for k2 in range(KT // 2):
                    nc.tensor.matmul(qps, w8_sb[:, 2, k2],
                                     xq8_sb[:, 1, 2 * k2:2 * k2 + 2, :],
                                     start=(k2 == 0), stop=(k2 == KT // 2 - 1),
                                     perf_mode=DR)
            nc.vector.tensor_copy(qT_sb[:, 512 * h:512 * h + 512], qps)

        def attend(J):
            m0 = 16 * J
            se = sepool.tile([P, R], bf16, tag="se", name=f"se{J}")
            se_by_J[J] = se
            sps = spool.tile([P, 1024], f32, tag="s", name=f"s{J}")
            if J < 32:
                nc.tensor.matmul(sps[:, m0:512], kT_sb[:, J, :],
                                 qT_sb[:, m0:512], start=True, stop=True)
                nc.tensor.matmul(sps[:, 512:1024], kT_sb[:, J, :],
                                 qT_sb[:, 512:1024], start=True, stop=True)
            else:
                nc.tensor.matmul(sps[:, m0:1024], kT_sb[:, J, :],
                                 qT_sb[:, m0:1024], start=True, stop=True)
            # ONE exp per key tile (reads across both PSUM banks)
            nc.scalar.activation(se[:, m0:1024], sps[:, m0:1024], Exp,
                                 scale=EXP_SCALE)
            nc.vector.tensor_mul(se[:, m0:m0 + 16], se[:, m0:m0 + 16],
                                 mask_sb)

        def attend_pv(J):
            m0 = 16 * J
            se = se_by_J.pop(J)
            if J < 32:
                nc.tensor.matmul(oT0[:, m0:512], v_sb[:, J, :], se[:, m0:512],
                                 start=(J == 0), stop=(J == 31))
                nc.tensor.matmul(oT1[:, :], v_sb[:, J, :], se[:, 512:1024],
                                 start=(J == 0), stop=False)
            else:
                c0 = m0 - 512
                nc.tensor.matmul(oT1[:, c0:512], v_sb[:, J, :],
                                 se[:, m0:1024], start=False, stop=(J == 63))
            with nc.allow_low_precision(reason="Z partials bf16"):
                nc.vector.tensor_add(zacc_sb[:, m0:1024],
                                     zacc_sb[:, m0:1024], se[:, m0:1024])

        def normalize(h):
            # Z per 128-query block: one [128,1] column per tiny matmul
            # zacc[:, blk].T @ ones; then a single DVE fast reciprocal.
            # No activation tables, no broadcast matmul: y is scaled by
            # 1/Z after projection via per-partition tensor_scalar.
            oT = (oT0, oT1)[h]
            c0 = 512 * h
            zps = pslo.tile([P, 512], f32, tag="lo", name=f"zps{h}")
            for qt in range(4):
                blk = slice(c0 + 128 * qt, c0 + 128 * qt + 128)
                nc.tensor.matmul(zps[:, qt:qt + 1], zacc_sb[:, blk],
                                 ones_sb, start=(qt == 0), stop=(qt == 3))
            nc.vector.reciprocal_approx_fast(zinv_sb[:, 4 * h:4 * h + 4],
                                             zps[:, 0:4])
            with nc.allow_low_precision(reason="O^T in bf16 for y matmul"):
                nc.vector.tensor_copy(oTb_sb[:, c0:c0 + 512], oT)

        def y_project(g):
            # y rows 128g..128g+127 (unnormalized, both d-halves in one
            # 2-bank PSUM tile), then one scale by 1/Z and one DMA.
            # Half 0 scales on DVE (ACT is mid-exp-stream); half 1 on ACT.
            yp = spool.tile([P, 1024], f32, tag="s", name=f"y{g}")
            for dc in range(2):
                nc.tensor.matmul(yp[:, 512 * dc:512 * dc + 512],
                                 oTb_sb[:, 128 * g:128 * g + 128],
                                 wo(dc), start=True, stop=True)
            y_sb = vpool.tile([P, 1024], bf16, tag="y", name=f"ysb{g}")
            with nc.allow_low_precision(reason="y output bf16"):
                if g < 4:
                    # mid-stream: ACT is busy with exps, scale on DVE
                    nc.vector.tensor_scalar_mul(y_sb, yp,
                                                zinv_sb[:, g:g + 1])
                else:
                    # tail: split halves across DVE + ACT in parallel
                    nc.vector.tensor_scalar_mul(y_sb[:, 0:512],
                                                yp[:, 0:512],
                                                zinv_sb[:, g:g + 1])
                    nc.scalar.activation(y_sb[:, 512:1024], yp[:, 512:1024],
                                         Copy, scale=zinv_sb[:, g:g + 1])
            if g < 4:
                eng = nc.gpsimd      # keep the sync ring free: it carries
            else:                    # the chunk-15 transpose these overlap
                eng = nc.scalar if g % 2 == 0 else nc.sync
            eng.dma_start(out=y[128 * g:128 * g + 128, :], in_=y_sb)

        se_by_J = {}
        # ---- main pipeline: project chunk ch, attend chunk ch-2 ----
        q = [load_chunk(1)]
        nc.sync.dma_start(out=wq_sb, in_=wqt[:, :, :])
        nc.scalar.dma_start(out=xqb_sb, in_=xqb[:, :, :])
        nc.gpsimd.dma_start(out=w8_sb, in_=w8[:, :, :, :, :])
        q.append(load_chunk(2))
        nc.scalar.dma_start(out=xq8_sb[:, 0], in_=xq8[:, 0])
        q.append(load_chunk(3))
        nc.scalar.dma_start(out=xq8_sb[:, 1], in_=xq8[:, 1])
        project_chunk0()
        # attend schedule: lag-2 through ch10, then drain the backlog with
        # 5-7 tiles per iteration so only J=60..63 trail the loop (a single
        # 12-tile iteration at ch15 left the PE idle around its edges)
        SCHED = {2: [0, 1], 3: [2, 3]}
        for c in range(4, 11):
            SCHED[c] = list(range(4 * (c - 3), 4 * (c - 3) + 4))
        SCHED[11] = list(range(32, 37))
        SCHED[12] = list(range(37, 42))
        SCHED[13] = list(range(42, 47))
        SCHED[14] = list(range(47, 53))
        SCHED[15] = list(range(53, 60))

        def attend_pairs(js):
            for i in range(0, len(js), 2):
                pair = js[i:i + 2]
                for J in pair:
                    attend(J)
                for J in pair:
                    attend_pv(J)

        for ch in range(1, NCH):
            xt_t = q.pop(0)
            if ch + 3 < NCH:
                q.append(load_chunk(ch + 3))
            project_chunk(ch, xt_t)
            # chunks <=12: transpose deferred one iteration (head-of-line
            # blocking); chunks >=13: inline (no more chunk loads to block)
            if ch <= 13:
                transpose_chunk(ch - 1)
            if ch >= 13:
                transpose_chunk(ch)
            if ch == 1:
                project_q(0)
                continue
            if ch == 2:
                project_q(1)
            attend_pairs(SCHED[ch])
            if ch == 10:
                normalize(0)
            elif 11 <= ch <= 14:
                y_project(ch - 11)  # half-0 output, spread over chunks
        attend_pairs([60, 61, 62, 63])
        normalize(1)
        for g in range(4, 8):
            y_project(g)

    nc.compile()
    return nc


def _get_nc():
    if "nc" not in _CACHE:
        _CACHE["nc"] = _build()
    return _CACHE["nc"]


def _host_in_maps(x, W_q, W_k, W_v, W_o):
    import ml_dtypes
    bf16 = ml_dtypes.bfloat16
    fp8 = ml_dtypes.float8_e4m3

    x = np.asarray(x, dtype=np.float32)
    xTf = np.ascontiguousarray(x.T)                        # [1024, 8192] f32
    xTk = xTf.reshape(KT, P, N_CTX)                        # [kt, p, col]
    # chunk 0 bf16: xt0[p, kt, j'] = xT[128kt+p, j']
    xt0_t = np.ascontiguousarray(
        xTk[:, :, 0:512].transpose(1, 0, 2).astype(bf16))
    # chunks 1..15 fp8: xt8[ch-1, p, kt, j'] = xT[128kt+p, 512ch + j']
    xt8_t = np.ascontiguousarray(
        xTk.reshape(KT, P, NCH, 512)[:, :, 1:].transpose(2, 1, 0, 3)
        .astype(fp8))

    def wtile(w, scale=1.0):
        # [p, kt, h] = scale * W[128kt + p, h]
        return (np.asarray(w, np.float32) * scale) \
            .reshape(KT, P, D_HEAD).transpose(1, 0, 2)

    def wtile8(w):
        # [p, k2, i, h] = 32*W[256k2 + 128i + p, h]  (fp8)
        return (np.asarray(w, np.float32) * SW).astype(fp8) \
            .reshape(KT // 2, 2, P, D_HEAD).transpose(2, 0, 1, 3)

    wkb_t = np.ascontiguousarray(wtile(W_k, SW).astype(bf16))
    wq_t = np.ascontiguousarray(wtile(W_q, SW).astype(bf16))
    # w8 = wk8 | wv8 | wq8 (all x32 fp8)
    w8_t = np.ascontiguousarray(
        np.stack([wtile8(W_k), wtile8(W_v), wtile8(W_q)], axis=1))
    # wp2 = wvb (x32) flat | wo (/32) | mask16 | ones
    wvb_flat = wtile(W_v, SW).reshape(P, KT * D_HEAD)
    wo_t = np.asarray(W_o, np.float32) / SW                # [128, 1024]
    ones_t = np.ones((P, 1), np.float32)
    pp = np.arange(P)[:, None]
    oo = np.arange(16)[None, :]

    in_maps = []
    for c in range(NCORES):
        # xq[p, ch, kt, m'] = xT[128kt+p, 8(512ch+m') + c]
        xq_c = xTk[:, :, c::NCORES] \
            .reshape(KT, P, 2, 512).transpose(1, 2, 0, 3)
        xq8_c = np.ascontiguousarray(xq_c.astype(fp8))
        xqb_c = np.ascontiguousarray(
            xq_c[:, 0, :, 0:64].astype(bf16))              # [P, KT, 64]
        mask_c = (8 * oo + c >= pp).astype(np.float32)     # [128, 16]
        wp2_t = np.ascontiguousarray(
            np.concatenate([wvb_flat, wo_t, mask_c, ones_t], axis=1)
            .astype(bf16))
        in_maps.append({
            "wkbt": wkb_t, "wqt": wq_t, "w8": w8_t, "wp2": wp2_t,
            "xt0": xt0_t, "xt8": xt8_t, "xq8": xq8_c, "xqb": xqb_c,
        })
    return in_maps


def _run(x, W_q, W_k, W_v, W_o, trace=False):
    from concourse.bass_utils import run_bass_kernel_spmd
    nc = _get_nc()
    in_maps = _host_in_maps(x, W_q, W_k, W_v, W_o)
    res = run_bass_kernel_spmd(nc, in_maps, list(range(NCORES)), trace=trace)
    out = np.empty((N_CTX, D_MODEL), dtype=np.float32)
    for c in range(NCORES):
        out[c::NCORES] = np.asarray(res.results[c]["y"], dtype=np.float32)
    return out, res


def kernel(x, W_q, W_k, W_v, W_o):
    out, _ = _run(x, W_q, W_k, W_v, W_o, trace=False)
    return out
